# revision 25
# baseline (speedup 1.0000x reference)
"""Multi-head attention (B=4, N=2048, d_model=1024, 16 heads) on 8 trn2 cores.

Sharding: data-parallel over batch (4) x Megatron tensor-parallel over heads
(2-way column-split Wq/Wk/Wv, row-split Wo).  Core c handles batch c//2 and
heads [8*(c%2), 8*(c%2)+8).  Each core emits a partial Y^T [1024, 2048]; the
host sums core pairs, transposes, and adds the output bias.  No on-device
collectives (a 2-rank 8MB AllReduce costs more than the whole compute).

On-device pipeline per core (bulk matmuls in bf16, fp32 PSUM accumulate;
the softmax-denominator path stays float32r):
  Q^T,K^T [512,2048] and V [2048, 8x(64+1)] projections (V gets a ones column
  so the attention-weight row sums fall out of the AV matmul), then per head
  pair: S^T = K_h @ Q_h^T (K=64 contraction, two heads packed concurrently in
  the PE array via base partitions 0/64), exp on the scalar engine with the
  1/sqrt(64) scale folded in, 0/1 mask multiply post-exp on GpSimd (causal
  variant touches diagonal tiles only and skips upper-triangle tiles), AV
  matmul (lhsT = V_aug) giving A^T plus the softmax denominators, and a
  selector-matmul broadcast of the batched reciprocals to normalize.
  Y^T = WoT^T @ A^T at the end.

Scheduling: everything is software-pipelined by emission order (engines run
in-order): scores run one kv-step ahead of AV; the next pair's Q/K projection
groups, the previous pair's normalization, and (during the last pair) the
finished t-blocks of the output projection are injected into the attention
kv-loop as paced PE filler so the tensor engine never idles long enough for
the HAM clock gate to re-throttle.
"""

import sys

for _p in ("/opt/trn_rl_repo",):
    if _p not in sys.path:
        sys.path.insert(0, _p)

from contextlib import ExitStack

import ml_dtypes
import numpy as np

import concourse.bass as bass
import concourse.mybir as mybir
import concourse.tile as tile_mod
from concourse.vector_clock import ScopedClock

# ---------------------------------------------------------------------------
# Workaround: this walrus build rejects >1 sync wait on a Drain (CTRL_NO)
# instruction ("Too many sync wait commands").  Tile's end-of-context drain
# carries one wait per live processor, so redistribute the extras onto
# individual EventSemaphore wait instructions.
# ---------------------------------------------------------------------------


def _patched_drain_and_barrier(self, tick_clock, wait_clock):
    nc = self.nc
    drain_inst = nc.sync.drain()
    wait_clock.add_sem_waits(
        drain_inst.ins, ScopedClock({None: tick_clock.global_clock})
    )
    si = drain_inst.ins.sync_info
    waits = list(si.on_wait) if si is not None else []
    if len(waits) > 1:
        assert self.sems is not None
        num2handle = {h.num: h for h in self.sems.allocated().values()}
        drain_inst.ins.sync_info = mybir.SyncInfo(
            on_wait=[waits[0]], on_update=list(si.on_update)
        )
        for w in waits[1:]:
            h = num2handle.get(w.id)
            assert h is not None, f"no sem handle for {w.ant_name} (id {w.id})"
            assert w.wait_mode.startswith("sem-ge"), w.wait_mode
            nc.sync.wait_ge(h, w.wait_value)

    nc.all_engine_barrier()
    assert self.sems is not None
    popped = nc._tile_sem_poison_stack.pop()
    assert popped is self._sem_poison
    nc.clear_and_free_semaphores(list(self.sems.allocated().values()))
    nc.all_engine_barrier()


tile_mod.TileContext._drain_and_barrier = _patched_drain_and_barrier


def _spill_excess_waits(nc: bass.Bass) -> None:
    """This walrus build accepts at most 1 sync wait per instruction (2 for
    EventSemaphore).  Move excess waits onto EventSemaphore instructions
    inserted just before the over-subscribed instruction on the same engine."""
    n_new = 0
    for f in nc.m.functions:
        for blk in f.blocks:
            il = blk.instructions
            out = []
            changed = False
            for inst in il:
                si = inst.sync_info
                waits = list(si.on_wait) if si is not None else []
                cap = 2 if isinstance(inst, mybir.InstEventSemaphore) else 1
                if len(waits) > cap:
                    changed = True
                    extra, keep = waits[:-cap], waits[-cap:]
                    inst.sync_info = mybir.SyncInfo(
                        on_wait=keep, on_update=list(si.on_update)
                    )
                    for j in range(0, len(extra), 2):
                        n_new += 1
                        out.append(
                            mybir.InstEventSemaphore(
                                name=f"{inst.name}-xw{j}",
                                ins=[],
                                outs=[],
                                engine=inst.engine,
                                sync_info=mybir.SyncInfo(
                                    on_wait=extra[j:j + 2], on_update=[]
                                ),
                            )
                        )
                out.append(inst)
            if changed:
                il[:] = out

# ---------------------------------------------------------------------------
# Problem shapes (hardcoded per the task contract).
# ---------------------------------------------------------------------------
B, N, D = 4, 2048, 1024
NHEAD, DEPTH = 16, 64
NCORES = 8
FH = 512          # features per core (8 heads x 64)
HPC = 8           # heads per core
P = 128           # SBUF partitions
TB = 512          # token block (matmul moving free dim)
NTB = N // TB     # 4 token blocks
KT = D // P       # 8 contraction tiles for the projections
NFT = FH // P     # 4 feature tiles (= head pairs)
NTT = N // P      # 16 token tiles
NKV = N // P      # 16 kv tiles
SCALE = 1.0 / np.sqrt(DEPTH)
F32, F32R, BF16 = mybir.dt.float32, mybir.dt.float32r, mybir.dt.bfloat16

_BF16 = ml_dtypes.bfloat16


def build_program(variant: str, zb: bool = False) -> bass.Bass:
    """variant: 'causal' (tril mask), 'full' (all-true mask), 'general'.
    zb: all-zero q/k/v biases (skip bias loads + adds on device)."""
    assert variant in ("causal", "full", "general")
    nc = bass.Bass()

    # pre-tiled on the host: partition-major layouts for fast (contiguous
    # per-partition) DMA
    xT = nc.declare_dram_parameter("xT", [P, KT, N], BF16, isOutput=False)
    wqT = nc.declare_dram_parameter("wqT", [NFT, P, KT, P], BF16, isOutput=False)
    wkT = nc.declare_dram_parameter("wkT", [NFT, P, KT, P], BF16, isOutput=False)
    wvT = nc.declare_dram_parameter("wvT", [P, KT, FH], BF16, isOutput=False)
    woT = nc.declare_dram_parameter("woT", [P, NFT, D], BF16, isOutput=False)
    bq2 = nc.declare_dram_parameter("bq2", [P, NFT], F32, isOutput=False)
    bk2 = nc.declare_dram_parameter("bk2", [P, NFT], F32, isOutput=False)
    bv1 = nc.declare_dram_parameter("bv1", [1, FH], BF16, isOutput=False)
    ones_d = nc.declare_dram_parameter("ones", [P, P], BF16, isOutput=False)
    # block-diagonal selector for broadcasting 1/sums rows (f32r path)
    sel_d = nc.declare_dram_parameter("sel", [8, 8 * DEPTH], F32, isOutput=False)
    if variant == "causal":
        # the 4 distinct diagonal-tile 0/1 patterns of the causal mask
        mb = nc.declare_dram_parameter("mb", [P, 4, TB], BF16, isOutput=False)
    elif variant == "general":
        mb = nc.declare_dram_parameter("mb", [P, NKV, NTB, TB], BF16, isOutput=False)
    # partials leave the device in bf16; the host sums the TP pair in f32
    yT = nc.declare_dram_parameter("yT", [D, N], BF16, isOutput=True)

    def nkv_of(qb):
        return 4 * (qb + 1) if variant == "causal" else NKV

    with tile_mod.TileContext(nc) as tc, ExitStack() as ctx:
        res = ctx.enter_context(tc.tile_pool(name="res", bufs=1))
        wp = ctx.enter_context(tc.tile_pool(name="w", bufs=2))
        pp = ctx.enter_context(tc.tile_pool(name="ppair", bufs=6))
        sums = ctx.enter_context(tc.tile_pool(name="sums", bufs=3))
        yst = ctx.enter_context(tc.tile_pool(name="yst", bufs=4))
        # PSUM: shared accumulator tag (2 banks) + score pair tiles (4) +
        # the two AV accumulators (2) = 8 banks exactly.
        mmp = ctx.enter_context(tc.tile_pool(name="mmp", bufs=2, space="PSUM"))
        pssp = ctx.enter_context(tc.tile_pool(name="pssp", bufs=2, space="PSUM"))
        psav = ctx.enter_context(tc.tile_pool(name="psav", bufs=1, space="PSUM"))
        if variant == "general":
            mgp = ctx.enter_context(tc.tile_pool(name="mg", bufs=4))

        if not zb:
            ones_sb = res.tile([P, P], BF16)
            bq_sb = res.tile([P, NFT], F32)
            bk_sb = res.tile([P, NFT], F32)
            bv_sb = res.tile([1, FH], BF16)
        sel_sb = res.tile([8, 8 * DEPTH], F32R)
        if variant == "causal":
            mb_sb = res.tile([P, 4, TB], BF16)

        qt_sb = res.tile([P, NFT, N], BF16)   # Q^T  [feat, tok]
        kt_sb = res.tile([P, NFT, N], BF16)   # K^T  [feat, tok]
        v_sb = res.tile([P, NTT, HPC, DEPTH + 1], BF16)  # V + ones col
        nc.gpsimd.memset(v_sb[:, :, :, DEPTH], 1.0)
        a_sb = res.tile([P, NFT, N], BF16)    # A^T (attention output)

        xt_sb = res.tile([P, KT, N], BF16)
        xT3 = xT
        wv_sb = res.tile([P, KT, FH], BF16)

        wq_sbs, wk_sbs = {}, {}

        def fetch_w(ft, ring=None):
            ring = ring or nc.sync
            wq_sbs[ft] = wp.tile([P, KT, P], BF16, tag="wq", name="wq_sb")
            wk_sbs[ft] = wp.tile([P, KT, P], BF16, tag="wk", name="wk_sb")
            ring.dma_start(wq_sbs[ft][:], wqT[ft])
            ring.dma_start(wk_sbs[ft][:], wkT[ft])

        def qk_drain(ft, tb, which, ps):
            ts = slice(tb * TB, (tb + 1) * TB)
            dst = qt_sb if which == "q" else kt_sb
            if zb:
                nc.vector.tensor_copy(dst[:, ft, ts], ps)
            else:
                bias = bq_sb if which == "q" else bk_sb
                nc.vector.tensor_tensor(
                    dst[:, ft, ts], ps,
                    bias[:, ft, None].to_broadcast((P, TB)),
                    mybir.AluOpType.add,
                )

        def emit_qk_group(ft, tb, which):
            ts = slice(tb * TB, (tb + 1) * TB)
            w_sb = wq_sbs[ft] if which == "q" else wk_sbs[ft]
            ps = mmp.tile([P, TB], F32, tag="acc", name="pqk")
            for k in range(KT):
                nc.tensor.matmul(
                    ps[:], w_sb[:, k, :], xt_sb[:, k, ts],
                    start=(k == 0), stop=(k == KT - 1),
                )
            qk_drain(ft, tb, which, ps[:])

        def qk_units(ft, tb, which):
            """One Q/K projection group split into 4 two-matmul filler units
            (shared PSUM accumulator; the last unit drains to SBUF)."""
            st = {}

            def u(j, ft=ft, tb=tb, which=which):
                if j == 0:
                    st["ps"] = mmp.tile([P, TB], F32, tag="acc", name="pqk")
                ps = st["ps"]
                w_sb = wq_sbs[ft] if which == "q" else wk_sbs[ft]
                ts = slice(tb * TB, (tb + 1) * TB)
                for k in (2 * j, 2 * j + 1):
                    nc.tensor.matmul(
                        ps[:], w_sb[:, k, :], xt_sb[:, k, ts],
                        start=(k == 0), stop=(k == KT - 1),
                    )
                if j == 3:
                    qk_drain(ft, tb, which, ps[:])

            return [lambda j=j: u(j) for j in range(4)]

        def proj_units(ft):
            out = []
            for tb in range(NTB):
                for which in ("q", "k"):
                    out += qk_units(ft, tb, which)
            return out

        def v_units(tt):
            """One V projection group split into 4 two-matmul units."""
            st = {}

            def u(j, tt=tt):
                if j == 0:
                    st["pv"] = mmp.tile([P, TB], F32, tag="acc", name="pv")
                pv = st["pv"]
                for k in (2 * j, 2 * j + 1):
                    nc.tensor.matmul(
                        pv[:], xt_sb[:, k, tt * P:(tt + 1) * P], wv_sb[:, k, :],
                        start=(k == 0), stop=(zb and k == KT - 1),
                    )
                if j == 3:
                    if not zb:
                        nc.tensor.matmul(
                            pv[:], ones_sb[0:1, 0:P], bv_sb[0:1, :],
                            start=False, stop=True,
                        )
                    nc.vector.tensor_copy(
                        v_sb[:, tt, :, 0:DEPTH],
                        pv[:].rearrange("p (h d) -> p h d", h=HPC),
                    )

            return [lambda j=j: u(j) for j in range(4)]

        def due_list(units, d0, d1):
            """Spread units evenly over due-steps [d0, d1)."""
            m = max(1, len(units))
            return [
                (d0 + (i * max(1, d1 - d0)) // m, u)
                for i, u in enumerate(units)
            ]

        def emit_v_group(tt):
            pv = mmp.tile([P, TB], F32, tag="acc", name="pv")
            for k in range(KT):
                nc.tensor.matmul(
                    pv[:], xt_sb[:, k, tt * P:(tt + 1) * P], wv_sb[:, k, :],
                    start=(k == 0), stop=(zb and k == KT - 1),
                )
            if not zb:
                nc.tensor.matmul(
                    pv[:], ones_sb[0:1, 0:P], bv_sb[0:1, :],
                    start=False, stop=True,
                )
            nc.vector.tensor_copy(
                v_sb[:, tt, :, 0:DEPTH],
                pv[:].rearrange("p (h d) -> p h d", h=HPC),
            )

        def emit_norm_qb(pr, rall, row0, qb):
            """bc2 = broadcast of 1/sums rows (2qb+hh) over the head depth;
            one in-place multiply normalizes both heads of the pair."""
            qs = slice(qb * TB, (qb + 1) * TB)
            nrows = rall.shape[0]
            bc = mmp.tile([P, TB], F32, tag="acc", name="bc")
            nc.tensor.matmul(
                bc[:],
                sel_sb[0:nrows, row0 * DEPTH:(row0 + 2) * DEPTH],
                rall[:],
                start=True, stop=True,
            )
            nc.vector.tensor_tensor(
                a_sb[:, pr, qs], a_sb[:, pr, qs], bc[:],
                mybir.AluOpType.mult,
            )

        sums_ps = {}
        rall_store = {}

        def norm_filler(pr):
            def recip(pr=pr, part=0):
                if part == 0:
                    rall_store[pr] = sums.tile([8, TB], F32R, tag="rall", name="rall")
                with nc.allow_low_precision(
                    reason="f32r holds fp32 bits; rounding happens in the PE"
                ):
                    cs = slice(part * P, (part + 1) * P)
                    nc.vector.reciprocal(
                        rall_store[pr][:, cs], sums_ps[pr][:, cs]
                    )

            for part in range(TB // P):
                yield lambda pr=pr, part=part: recip(pr, part)
            for qb in range(NTB):
                yield lambda pr=pr, qb=qb: emit_norm_qb(
                    pr, rall_store[pr], 2 * qb, qb
                )

        wo_sb = res.tile([P, NFT, D], BF16)
        yT3 = yT.rearrange("(o p) t -> p o t", p=P)

        def emit_outproj_group(ot, tb):
            ts = slice(tb * TB, (tb + 1) * TB)
            py = mmp.tile([P, TB], F32, tag="acc", name="py")
            for f in range(NFT):
                nc.tensor.matmul(
                    py[:], wo_sb[:, f, ot * P:(ot + 1) * P],
                    a_sb[:, f, ts],
                    start=(f == 0), stop=(f == NFT - 1),
                )
            yt = yst.tile([P, TB], BF16, tag="yt")
            nc.vector.tensor_copy(yt[:], py[:])
            nc.sync.dma_start(yT3[:, ot, ts], yt[:])

        def attention(pr, sched, qb_prologue=None):
            """sched: [(due_step, fn)] sorted by due-step; per kv-step all
            due units run (~1/step keeps a short PE burst between the AV pair
            and the next score pair so the score weight loads hide).  Scores
            run one step ahead of AV across qb boundaries."""
            last = pr == NFT - 1
            if not last:
                sums_p = sums.tile([8, TB], F32, tag="sums_p", name="sums_p")
                sums_ps[pr] = sums_p
            steps = [(qb, kv) for qb in range(NTB) for kv in range(nkv_of(qb))]
            kvstep = 0
            avs = {}
            s3s = {}
            pending = []  # scores run 2 kv-steps ahead of AV (exp+mask slack)

            def qb_end(qb):
                qs = slice(qb * TB, (qb + 1) * TB)
                av = avs.pop(qb)
                for hh in (0, 1):
                    srow = sums.tile([P, TB], F32, tag="srow", name="srow")
                    nc.vector.tensor_copy(
                        srow[DEPTH:DEPTH + 1, :], av[hh][DEPTH:DEPTH + 1, :]
                    )
                    if last:
                        nc.sync.dma_start(
                            s3s[qb][hh:hh + 1, :], srow[DEPTH:DEPTH + 1, :]
                        )
                    else:
                        nc.sync.dma_start(
                            sums_p[2 * qb + hh:2 * qb + hh + 1, :],
                            srow[DEPTH:DEPTH + 1, :],
                        )
                    nc.vector.tensor_copy(
                        a_sb[64 * hh:64 * hh + 64, pr, qs], av[hh][0:DEPTH, :]
                    )
                if last:
                    s3 = s3s.pop(qb)

                    rall3 = [None]

                    def norm3_part(part, qb=qb, s3=s3, rall3=rall3):
                        if part == 0:
                            rall3[0] = sums.tile([2, TB], F32R, tag="r3", name="rall3")
                        with nc.allow_low_precision(
                            reason="f32r holds fp32 bits; PE does the rounding"
                        ):
                            cs = slice(part * P, (part + 1) * P)
                            nc.vector.reciprocal(rall3[0][:, cs], s3[:, cs])
                        if part == TB // P - 1:
                            emit_norm_qb(pr, rall3[0], 0, qb)

                    for part in range(TB // P):
                        deferred.append(lambda part=part: norm3_part(part))
                    for ot in range(D // P):
                        deferred.append(
                            lambda ot=ot, tb=qb: emit_outproj_group(ot, tb)
                        )

            def av_step(pqb, pkv, ppt, poff):
                for hh in (0, 1):
                    nc.tensor.matmul(
                        avs[pqb][hh][0:DEPTH + 1, poff:TB],
                        v_sb[:, pkv, 2 * pr + hh, :],
                        ppt[:, hh * TB + poff:(hh + 1) * TB],
                        start=(pkv == 0), stop=(pkv == nkv_of(pqb) - 1),
                    )

            for qb, kv in steps:
                if kv == 0:
                    if qb_prologue is not None:
                        qb_prologue(qb)
                    if last:
                        s3s[qb] = sums.tile([2, TB], F32, tag="s3", name="s3")
                # causal diagonal tiles: tokens below the kv tile can't
                # attend, so the score/exp/AV column window shrinks to
                # [i*P, TB) and only the 128-wide triangle block is masked
                di = kv - 4 * qb if (variant == "causal" and kv >= 4 * qb) else -1
                off = di * P if di > 0 else 0
                qs = slice(qb * TB + off, (qb + 1) * TB)
                sp = pssp.tile([P, 2 * TB], F32, tag="sp")
                for hh in (0, 1):
                    hs = slice(64 * hh, 64 * hh + 64)
                    nc.tensor.matmul(
                        sp[:, hh * TB + off:(hh + 1) * TB],
                        kt_sb[hs, pr, kv * P:(kv + 1) * P],
                        qt_sb[hs, pr, qs],
                        start=True, stop=True,
                    )
                pt = pp.tile([P, 2 * TB], BF16, tag="pt")
                if off:
                    for hh in (0, 1):
                        nc.scalar.activation(
                            pt[:, hh * TB + off:(hh + 1) * TB],
                            sp[:, hh * TB + off:(hh + 1) * TB],
                            mybir.ActivationFunctionType.Exp,
                            scale=float(SCALE),
                        )
                else:
                    nc.scalar.activation(
                        pt[:], sp[:], mybir.ActivationFunctionType.Exp,
                        scale=float(SCALE),
                    )
                # mask applied post-exp as a 0/1 multiply on the idle GpSimd
                # engine (SBUF-only), keeping the DVE off the exp->AV chain
                if di >= 0:
                    ms = slice(di * P, (di + 1) * P)
                    for hh in (0, 1):
                        nc.gpsimd.tensor_tensor(
                            pt[:, hh * TB + di * P:hh * TB + (di + 1) * P],
                            pt[:, hh * TB + di * P:hh * TB + (di + 1) * P],
                            mb_sb[:, di, ms], mybir.AluOpType.mult,
                        )
                elif variant == "general":
                    mg = mgp.tile([P, TB], BF16, tag="mg")
                    nc.sync.dma_start(mg[:], mb[:, kv, qb, :])
                    for hh in (0, 1):
                        nc.gpsimd.tensor_tensor(
                            pt[:, hh * TB:(hh + 1) * TB],
                            pt[:, hh * TB:(hh + 1) * TB],
                            mg[:], mybir.AluOpType.mult,
                        )
                pending.append((qb, kv, pt, off))
                if len(pending) > 2:
                    pqb, pkv, ppt, poff = pending.pop(0)
                    if pkv == 0:
                        avs[pqb] = [
                            psav.tile([P, TB], F32, tag=f"av{h}", name=f"av{h}")
                            for h in (0, 1)
                        ]
                    av_step(pqb, pkv, ppt, poff)
                    if pkv == nkv_of(pqb) - 1:
                        qb_end(pqb)
                kvstep += 1
                ran = False
                while sched and sched[0][0] <= kvstep:
                    sched.pop(0)[1]()
                    ran = True
                if deferred and not ran:
                    deferred.pop(0)()
                    if deferred and len(deferred) > 8:
                        deferred.pop(0)()
                elif deferred and len(deferred) > 24:
                    deferred.pop(0)()
            # drain the pipeline
            for pqb, pkv, ppt, poff in pending:
                if pkv == 0:
                    avs[pqb] = [
                        psav.tile([P, TB], F32, tag=f"av{h}", name=f"av{h}")
                        for h in (0, 1)
                    ]
                av_step(pqb, pkv, ppt, poff)
                if pkv == nkv_of(pqb) - 1:
                    qb_end(pqb)
            for _, u in sched:
                u()

        # ---- schedule ----------------------------------------------------
        deferred = []
        # Startup: xT arrives as 8 per-k-tile chunks round-robined over the
        # sync/scalar/vector DMA rings (k ascending); ft0's weights lead the
        # gpsimd ring so the first matmul can fire as soon as chunk 0 lands.
        if not zb:
            nc.sync.dma_start(bq_sb[:], bq2[:])
            nc.sync.dma_start(bk_sb[:], bk2[:])
        fetch_w(0, ring=nc.gpsimd)
        # xt k-chunks across the three rings; per-ring transfers run
        # sequentially (~143 GB/s each), so chunk k's arrival time is its
        # queue position -- k0 is split in half across two rings so the
        # first projection matmuls can fire ~2us earlier
        nc.sync.dma_start(xt_sb[:, 0, 0:N // 2], xT3[:, 0, 0:N // 2])
        nc.scalar.dma_start(xt_sb[:, 0, N // 2:N], xT3[:, 0, N // 2:N])
        nc.sync.dma_start(xt_sb[:, 1, :], xT3[:, 1, :])
        nc.scalar.dma_start(xt_sb[:, 2, :], xT3[:, 2, :])
        nc.gpsimd.dma_start(xt_sb[:, 4, :], xT3[:, 4, :])
        nc.sync.dma_start(xt_sb[:, 3, :], xT3[:, 3, :])
        nc.scalar.dma_start(xt_sb[:, 5, :], xT3[:, 5, :])
        nc.gpsimd.dma_start(xt_sb[:, 7, :], xT3[:, 7, :])
        nc.sync.dma_start(xt_sb[:, 6, :], xT3[:, 6, :])
        nc.gpsimd.dma_start(wv_sb[:], wvT[:])
        if variant == "causal":
            nc.gpsimd.dma_start(mb_sb[:], mb[:])
        fetch_w(1, ring=nc.gpsimd)
        if not zb:
            nc.scalar.dma_start(ones_sb[:], ones_d[:])
            nc.scalar.dma_start(bv_sb[:], bv1[:])
        nc.scalar.dma_start(sel_sb[:], sel_d[:].bitcast(F32R))

        # ft0's eight Q/K projection groups run k-interleaved across all 8
        # PSUM banks so the PE tracks the xT chunk arrivals instead of
        # stalling on the full tensor.
        st_groups = [(tb, w) for w in ("q", "k") for tb in range(NTB)]
        st_acc = [mmp.tile([P, TB], F32, tag="acc", name="pqk0") for _ in range(2)]
        st_sp = [pssp.tile([P, 2 * TB], F32, tag="sp", name="sp0") for _ in range(2)]
        st_av = [psav.tile([P, TB], F32, tag=f"av{h}", name=f"av{h}0") for h in (0, 1)]
        st_slots = [
            st_acc[0][:, :], st_acc[1][:, :],
            st_sp[0][:, 0:TB], st_sp[0][:, TB:2 * TB],
            st_sp[1][:, 0:TB], st_sp[1][:, TB:2 * TB],
            st_av[0][:, :], st_av[1][:, :],
        ]
        korder = (0, 4, 1, 2, 3, 7, 5, 6)  # xt chunk DMA arrival order
        for ki, k in enumerate(korder):
            for (tb, which), ps in zip(st_groups, st_slots):
                w_sb = wq_sbs[0] if which == "q" else wk_sbs[0]
                nc.tensor.matmul(
                    ps, w_sb[:, k, :], xt_sb[:, k, tb * TB:(tb + 1) * TB],
                    start=(ki == 0), stop=(ki == KT - 1),
                )
        # drain in (tb, q/k) order so the first attention steps unblock first
        st_by_key = dict(zip(st_groups, st_slots))
        for tb in range(NTB):
            for which in ("q", "k"):
                qk_drain(0, tb, which, st_by_key[(tb, which)])

        def v_prologue(qb):
            # qb0's V tiles inline; later qbs' V groups ride the filler sched
            if variant == "causal":
                tts = range(4) if qb == 0 else ()
            else:
                tts = range(NTT) if qb == 0 else ()
            for tt in tts:
                emit_v_group(tt)

        ns = sum(nkv_of(qb) for qb in range(NTB))
        s0 = []
        if variant == "causal":
            for qb in (1, 2, 3):
                vu = []
                for tt in range(4 * qb, 4 * qb + 4):
                    vu += v_units(tt)
                # due before qb's steps begin (qb's AV starts one step in)
                d1 = sum(nkv_of(q) for q in range(qb))
                s0 += due_list(vu, d1 - 10 if qb > 1 else 0, d1)
            s0 += due_list(proj_units(1), 24, ns)
        else:
            s0 += due_list(proj_units(1), 4, ns)
        s0.sort(key=lambda t: t[0])
        attention(0, s0, qb_prologue=v_prologue)
        fetch_w(2)
        p2 = proj_units(2)
        s1 = due_list(p2, 1, ns) + due_list(list(norm_filler(0)), 4, 12)
        s1.sort(key=lambda t: t[0])
        attention(1, s1)
        fetch_w(3)
        p3 = proj_units(3)
        s2 = due_list(p3, 1, ns) + due_list(list(norm_filler(1)), 4, 12)
        s2.sort(key=lambda t: t[0])
        attention(2, s2)
        nc.sync.dma_start(wo_sb[:], woT[:])

        attention(3, due_list(list(norm_filler(2)), 4, 12))
        while deferred:
            deferred.pop(0)()

    _spill_excess_waits(nc)
    return nc


# ---------------------------------------------------------------------------
# Host side
# ---------------------------------------------------------------------------
_cache: dict[tuple, bass.Bass] = {}


def _get_program(variant: str, zb: bool) -> bass.Bass:
    key = (variant, zb)
    if key not in _cache:
        _cache[key] = build_program(variant, zb)
    return _cache[key]


def _mask_variant(mask: np.ndarray) -> str:
    if mask.all():
        return "full"
    if np.array_equal(mask, np.tril(np.ones_like(mask))):
        return "causal"
    return "general"


def _make_in_maps(input, mask, Wq, bq, Wk, bk, Wv, bv, Wo, bo, variant):
    input = np.asarray(input, np.float32)
    mask = np.asarray(mask, bool)
    Wq, Wk, Wv, Wo = (np.asarray(w, np.float32) for w in (Wq, Wk, Wv, Wo))
    bq, bk, bv = (np.asarray(b, np.float32) for b in (bq, bk, bv))
    sel = np.kron(np.eye(8, dtype=np.float32), np.ones((1, DEPTH), np.float32))

    mb_arrs = {}
    if variant != "full":
        # 0/1 multiplicative mask on P = exp(S^T) (applied post-exp)
        maskT01 = mask.T.astype(np.float32)
        if variant == "causal":
            # the diag-tile pattern only depends on kv-tile offset within the
            # 512-block, so 4 patterns cover all q blocks
            mb = np.empty((P, 4, TB), _BF16)
            for i in range(4):
                mb[:, i, :] = maskT01[i * P:(i + 1) * P, 0:TB]
        else:
            mb = (
                maskT01.reshape(NKV, P, NTB, TB)
                .transpose(1, 0, 2, 3)
                .astype(_BF16)
            )
        mb_arrs["mb"] = np.ascontiguousarray(mb)

    in_maps = []
    for c in range(NCORES):
        b, half = c // 2, c % 2
        fs = FH * half
        def tile_kp(wt):
            # [D, F] -> [P, KT, F] with row 128k+p -> [p, k]
            return wt.reshape(KT, P, -1).transpose(1, 0, 2)

        def tile_ft(wt):
            # [D, FH] -> [NFT, P, KT, P]: per f-tile, [p, k, f]
            return np.stack(
                [tile_kp(wt[:, ft * P:(ft + 1) * P]) for ft in range(NFT)]
            )

        m = {
            "xT": np.ascontiguousarray(tile_kp(input[b].T.astype(_BF16))),
            "wqT": np.ascontiguousarray(tile_ft(Wq[fs:fs + FH, :].T.astype(_BF16))),
            "wkT": np.ascontiguousarray(tile_ft(Wk[fs:fs + FH, :].T.astype(_BF16))),
            "wvT": np.ascontiguousarray(tile_kp(Wv[fs:fs + FH, :].T.astype(_BF16))),
            "woT": np.ascontiguousarray(
                Wo[:, fs:fs + FH].T.astype(_BF16).reshape(NFT, P, D).transpose(1, 0, 2)
            ),
            "bq2": np.ascontiguousarray(bq[fs:fs + FH].reshape(NFT, P).T),
            "bk2": np.ascontiguousarray(bk[fs:fs + FH].reshape(NFT, P).T),
            "bv1": np.ascontiguousarray(bv[fs:fs + FH].reshape(1, FH).astype(_BF16)),
            "ones": np.ones((P, P), _BF16),
            "sel": sel,
        }
        m.update(mb_arrs)
        in_maps.append(m)
    return in_maps


def _run(inputs: dict, trace: bool = False, tmpdir=None):
    from concourse.bass_utils import run_bass_kernel_spmd

    variant = _mask_variant(np.asarray(inputs["mask"], bool))
    zb = all(
        not np.any(np.asarray(inputs[k], np.float32))
        for k in ("bq", "bk", "bv")
    )
    nc = _get_program(variant, zb)
    in_maps = _make_in_maps(
        inputs["input"], inputs["mask"],
        inputs["Wq"], inputs["bq"], inputs["Wk"], inputs["bk"],
        inputs["Wv"], inputs["bv"], inputs["Wo"], inputs["bo"],
        variant,
    )
    res = run_bass_kernel_spmd(
        nc, in_maps, list(range(NCORES)), trace=trace, tmpdir=tmpdir
    )
    bo = np.asarray(inputs["bo"], np.float32)
    out = np.empty((B, N, D), np.float32)
    for b in range(B):
        yT = (
            res.results[2 * b]["yT"].astype(np.float32)
            + res.results[2 * b + 1]["yT"].astype(np.float32)
        )
        out[b] = yT.T + bo
    return out, res


def kernel(**inputs) -> np.ndarray:
    out, _ = _run(inputs, trace=False)
    return out



# revision 28
# speedup vs baseline: 1.0203x; 1.0203x over previous
"""Multi-head attention (B=4, N=2048, d_model=1024, 16 heads) on 8 trn2 cores.

Sharding: data-parallel over batch (4) x Megatron tensor-parallel over heads
(2-way column-split Wq/Wk/Wv, row-split Wo).  Core c handles batch c//2 and
heads [8*(c%2), 8*(c%2)+8).  Each core emits a partial Y^T [1024, 2048]; the
host sums core pairs, transposes, and adds the output bias.  No on-device
collectives (a 2-rank 8MB AllReduce costs more than the whole compute).

On-device pipeline per core (bulk matmuls in bf16, fp32 PSUM accumulate;
the softmax-denominator path stays float32r):
  Q^T,K^T [512,2048] and V [2048, 8x(64+1)] projections (V gets a ones column
  so the attention-weight row sums fall out of the AV matmul), then per head
  pair: S^T = K_h @ Q_h^T (K=64 contraction, two heads packed concurrently in
  the PE array via base partitions 0/64), exp on the scalar engine with the
  1/sqrt(64) scale folded in, 0/1 mask multiply post-exp on GpSimd (causal
  variant touches diagonal tiles only and skips upper-triangle tiles), AV
  matmul (lhsT = V_aug) giving A^T plus the softmax denominators, and a
  selector-matmul broadcast of the batched reciprocals to normalize.
  Y^T = WoT^T @ A^T at the end.

Scheduling: everything is software-pipelined by emission order (engines run
in-order): scores run one kv-step ahead of AV; the next pair's Q/K projection
groups, the previous pair's normalization, and (during the last pair) the
finished t-blocks of the output projection are injected into the attention
kv-loop as paced PE filler so the tensor engine never idles long enough for
the HAM clock gate to re-throttle.
"""

import sys

for _p in ("/opt/trn_rl_repo",):
    if _p not in sys.path:
        sys.path.insert(0, _p)

from contextlib import ExitStack

import ml_dtypes
import numpy as np

import concourse.bass as bass
import concourse.mybir as mybir
import concourse.tile as tile_mod
from concourse.vector_clock import ScopedClock

# ---------------------------------------------------------------------------
# Workaround: this walrus build rejects >1 sync wait on a Drain (CTRL_NO)
# instruction ("Too many sync wait commands").  Tile's end-of-context drain
# carries one wait per live processor, so redistribute the extras onto
# individual EventSemaphore wait instructions.
# ---------------------------------------------------------------------------


def _patched_drain_and_barrier(self, tick_clock, wait_clock):
    nc = self.nc
    drain_inst = nc.sync.drain()
    wait_clock.add_sem_waits(
        drain_inst.ins, ScopedClock({None: tick_clock.global_clock})
    )
    si = drain_inst.ins.sync_info
    waits = list(si.on_wait) if si is not None else []
    if len(waits) > 1:
        assert self.sems is not None
        num2handle = {h.num: h for h in self.sems.allocated().values()}
        drain_inst.ins.sync_info = mybir.SyncInfo(
            on_wait=[waits[0]], on_update=list(si.on_update)
        )
        for w in waits[1:]:
            h = num2handle.get(w.id)
            assert h is not None, f"no sem handle for {w.ant_name} (id {w.id})"
            assert w.wait_mode.startswith("sem-ge"), w.wait_mode
            nc.sync.wait_ge(h, w.wait_value)

    nc.all_engine_barrier()
    assert self.sems is not None
    popped = nc._tile_sem_poison_stack.pop()
    assert popped is self._sem_poison
    nc.clear_and_free_semaphores(list(self.sems.allocated().values()))
    nc.all_engine_barrier()


tile_mod.TileContext._drain_and_barrier = _patched_drain_and_barrier


def _spill_excess_waits(nc: bass.Bass) -> None:
    """This walrus build accepts at most 1 sync wait per instruction (2 for
    EventSemaphore).  Move excess waits onto EventSemaphore instructions
    inserted just before the over-subscribed instruction on the same engine."""
    n_new = 0
    for f in nc.m.functions:
        for blk in f.blocks:
            il = blk.instructions
            out = []
            changed = False
            for inst in il:
                si = inst.sync_info
                waits = list(si.on_wait) if si is not None else []
                cap = 2 if isinstance(inst, mybir.InstEventSemaphore) else 1
                if len(waits) > cap:
                    changed = True
                    extra, keep = waits[:-cap], waits[-cap:]
                    inst.sync_info = mybir.SyncInfo(
                        on_wait=keep, on_update=list(si.on_update)
                    )
                    for j in range(0, len(extra), 2):
                        n_new += 1
                        out.append(
                            mybir.InstEventSemaphore(
                                name=f"{inst.name}-xw{j}",
                                ins=[],
                                outs=[],
                                engine=inst.engine,
                                sync_info=mybir.SyncInfo(
                                    on_wait=extra[j:j + 2], on_update=[]
                                ),
                            )
                        )
                out.append(inst)
            if changed:
                il[:] = out

# ---------------------------------------------------------------------------
# Problem shapes (hardcoded per the task contract).
# ---------------------------------------------------------------------------
B, N, D = 4, 2048, 1024
NHEAD, DEPTH = 16, 64
NCORES = 8
FH = 512          # features per core (8 heads x 64)
HPC = 8           # heads per core
P = 128           # SBUF partitions
TB = 512          # token block (matmul moving free dim)
NTB = N // TB     # 4 token blocks
KT = D // P       # 8 contraction tiles for the projections
NFT = FH // P     # 4 feature tiles (= head pairs)
NTT = N // P      # 16 token tiles
NKV = N // P      # 16 kv tiles
SCALE = 1.0 / np.sqrt(DEPTH)
F32, F32R, BF16 = mybir.dt.float32, mybir.dt.float32r, mybir.dt.bfloat16

_BF16 = ml_dtypes.bfloat16


def build_program(variant: str, zb: bool = False) -> bass.Bass:
    """variant: 'causal' (tril mask), 'full' (all-true mask), 'general'.
    zb: all-zero q/k/v biases (skip bias loads + adds on device)."""
    assert variant in ("causal", "full", "general")
    nc = bass.Bass()

    # pre-tiled on the host: partition-major layouts for fast (contiguous
    # per-partition) DMA
    xT = nc.declare_dram_parameter("xT", [P, KT, N], BF16, isOutput=False)
    wqT = nc.declare_dram_parameter("wqT", [NFT, P, KT, P], BF16, isOutput=False)
    wkT = nc.declare_dram_parameter("wkT", [NFT, P, KT, P], BF16, isOutput=False)
    wvT = nc.declare_dram_parameter("wvT", [P, KT, FH], BF16, isOutput=False)
    woT = nc.declare_dram_parameter("woT", [P, NFT, D], BF16, isOutput=False)
    bq2 = nc.declare_dram_parameter("bq2", [P, NFT], F32, isOutput=False)
    bk2 = nc.declare_dram_parameter("bk2", [P, NFT], F32, isOutput=False)
    bv1 = nc.declare_dram_parameter("bv1", [1, FH], BF16, isOutput=False)
    ones_d = nc.declare_dram_parameter("ones", [P, P], BF16, isOutput=False)
    # block-diagonal selector for broadcasting 1/sums rows (f32r path)
    sel_d = nc.declare_dram_parameter("sel", [8, 8 * DEPTH], F32, isOutput=False)
    if variant == "causal":
        # the 4 distinct diagonal-tile 0/1 patterns of the causal mask
        mb = nc.declare_dram_parameter("mb", [P, 4, TB], BF16, isOutput=False)
    elif variant == "general":
        mb = nc.declare_dram_parameter("mb", [P, NKV, NTB, TB], BF16, isOutput=False)
    # partials leave the device in bf16; the host sums the TP pair in f32
    yT = nc.declare_dram_parameter("yT", [D, N], BF16, isOutput=True)

    def nkv_of(qb):
        return 4 * (qb + 1) if variant == "causal" else NKV

    with tile_mod.TileContext(nc) as tc, ExitStack() as ctx:
        res = ctx.enter_context(tc.tile_pool(name="res", bufs=1))
        wp = ctx.enter_context(tc.tile_pool(name="w", bufs=2))
        pp = ctx.enter_context(tc.tile_pool(name="ppair", bufs=6))
        sums = ctx.enter_context(tc.tile_pool(name="sums", bufs=3))
        yst = ctx.enter_context(tc.tile_pool(name="yst", bufs=4))
        # PSUM: shared accumulator tag (2 banks) + score pair tiles (4) +
        # the two AV accumulators (2) = 8 banks exactly.
        mmp = ctx.enter_context(tc.tile_pool(name="mmp", bufs=2, space="PSUM"))
        pssp = ctx.enter_context(tc.tile_pool(name="pssp", bufs=2, space="PSUM"))
        psav = ctx.enter_context(tc.tile_pool(name="psav", bufs=1, space="PSUM"))
        if variant == "general":
            mgp = ctx.enter_context(tc.tile_pool(name="mg", bufs=4))

        if not zb:
            ones_sb = res.tile([P, P], BF16)
            bq_sb = res.tile([P, NFT], F32)
            bk_sb = res.tile([P, NFT], F32)
            bv_sb = res.tile([1, FH], BF16)
        sel_sb = res.tile([8, 8 * DEPTH], F32R)
        if variant == "causal":
            mb_sb = res.tile([P, 4, TB], BF16)

        qt_sb = res.tile([P, NFT, N], BF16)   # Q^T  [feat, tok]
        kt_sb = res.tile([P, NFT, N], BF16)   # K^T  [feat, tok]
        v_sb = res.tile([P, NTT, HPC, DEPTH + 1], BF16)  # V + ones col
        nc.gpsimd.memset(v_sb[:, :, :, DEPTH], 1.0)
        a_sb = res.tile([P, NFT, N], BF16)    # A^T (attention output)

        xt_sb = res.tile([P, KT, N], BF16)
        xT3 = xT
        wv_sb = res.tile([P, KT, FH], BF16)

        wq_sbs, wk_sbs = {}, {}

        def fetch_w(ft, ring=None):
            ring = ring or nc.sync
            wq_sbs[ft] = wp.tile([P, KT, P], BF16, tag="wq", name="wq_sb")
            wk_sbs[ft] = wp.tile([P, KT, P], BF16, tag="wk", name="wk_sb")
            ring.dma_start(wq_sbs[ft][:], wqT[ft])
            ring.dma_start(wk_sbs[ft][:], wkT[ft])

        def qk_drain(ft, tb, which, ps):
            ts = slice(tb * TB, (tb + 1) * TB)
            dst = qt_sb if which == "q" else kt_sb
            if zb:
                nc.vector.tensor_copy(dst[:, ft, ts], ps)
            else:
                bias = bq_sb if which == "q" else bk_sb
                nc.vector.tensor_tensor(
                    dst[:, ft, ts], ps,
                    bias[:, ft, None].to_broadcast((P, TB)),
                    mybir.AluOpType.add,
                )

        def emit_qk_group(ft, tb, which):
            ts = slice(tb * TB, (tb + 1) * TB)
            w_sb = wq_sbs[ft] if which == "q" else wk_sbs[ft]
            ps = mmp.tile([P, TB], F32, tag="acc", name="pqk")
            for k in range(KT):
                nc.tensor.matmul(
                    ps[:], w_sb[:, k, :], xt_sb[:, k, ts],
                    start=(k == 0), stop=(k == KT - 1),
                )
            qk_drain(ft, tb, which, ps[:])

        def qk_units(ft, tb, which):
            """One Q/K projection group split into 4 two-matmul filler units
            (shared PSUM accumulator; the last unit drains to SBUF)."""
            st = {}

            def u(j, ft=ft, tb=tb, which=which):
                if j == 0:
                    st["ps"] = mmp.tile([P, TB], F32, tag="acc", name="pqk")
                ps = st["ps"]
                w_sb = wq_sbs[ft] if which == "q" else wk_sbs[ft]
                ts = slice(tb * TB, (tb + 1) * TB)
                for k in (2 * j, 2 * j + 1):
                    nc.tensor.matmul(
                        ps[:], w_sb[:, k, :], xt_sb[:, k, ts],
                        start=(k == 0), stop=(k == KT - 1),
                    )
                if j == 3:
                    qk_drain(ft, tb, which, ps[:])

            return [lambda j=j: u(j) for j in range(4)]

        def proj_units(ft):
            out = []
            for tb in range(NTB):
                for which in ("q", "k"):
                    out += qk_units(ft, tb, which)
            return out

        def v_units(tt):
            """One V projection group split into 4 two-matmul units."""
            st = {}

            def u(j, tt=tt):
                if j == 0:
                    st["pv"] = mmp.tile([P, TB], F32, tag="acc", name="pv")
                pv = st["pv"]
                for k in (2 * j, 2 * j + 1):
                    nc.tensor.matmul(
                        pv[:], xt_sb[:, k, tt * P:(tt + 1) * P], wv_sb[:, k, :],
                        start=(k == 0), stop=(zb and k == KT - 1),
                    )
                if j == 3:
                    if not zb:
                        nc.tensor.matmul(
                            pv[:], ones_sb[0:1, 0:P], bv_sb[0:1, :],
                            start=False, stop=True,
                        )
                    nc.vector.tensor_copy(
                        v_sb[:, tt, :, 0:DEPTH],
                        pv[:].rearrange("p (h d) -> p h d", h=HPC),
                    )

            return [lambda j=j: u(j) for j in range(4)]

        def due_list(units, d0, d1):
            """Spread units evenly over due-steps [d0, d1)."""
            m = max(1, len(units))
            return [
                (d0 + (i * max(1, d1 - d0)) // m, u)
                for i, u in enumerate(units)
            ]

        def emit_v_group(tt):
            pv = mmp.tile([P, TB], F32, tag="acc", name="pv")
            for k in range(KT):
                nc.tensor.matmul(
                    pv[:], xt_sb[:, k, tt * P:(tt + 1) * P], wv_sb[:, k, :],
                    start=(k == 0), stop=(zb and k == KT - 1),
                )
            if not zb:
                nc.tensor.matmul(
                    pv[:], ones_sb[0:1, 0:P], bv_sb[0:1, :],
                    start=False, stop=True,
                )
            nc.vector.tensor_copy(
                v_sb[:, tt, :, 0:DEPTH],
                pv[:].rearrange("p (h d) -> p h d", h=HPC),
            )

        def emit_norm_qb(pr, rall, row0, qb):
            """bc2 = broadcast of 1/sums rows (2qb+hh) over the head depth;
            one in-place multiply normalizes both heads of the pair."""
            qs = slice(qb * TB, (qb + 1) * TB)
            nrows = rall.shape[0]
            bc = mmp.tile([P, TB], F32, tag="acc", name="bc")
            nc.tensor.matmul(
                bc[:],
                sel_sb[0:nrows, row0 * DEPTH:(row0 + 2) * DEPTH],
                rall[:],
                start=True, stop=True,
            )
            nc.vector.tensor_tensor(
                a_sb[:, pr, qs], a_sb[:, pr, qs], bc[:],
                mybir.AluOpType.mult,
            )

        sums_ps = {}
        rall_store = {}

        def norm_filler(pr):
            def recip(pr=pr, part=0):
                if part == 0:
                    rall_store[pr] = sums.tile([8, TB], F32R, tag="rall", name="rall")
                with nc.allow_low_precision(
                    reason="f32r holds fp32 bits; rounding happens in the PE"
                ):
                    cs = slice(part * P, (part + 1) * P)
                    nc.vector.reciprocal(
                        rall_store[pr][:, cs], sums_ps[pr][:, cs]
                    )

            for part in range(TB // P):
                yield lambda pr=pr, part=part: recip(pr, part)
            for qb in range(NTB):
                yield lambda pr=pr, qb=qb: emit_norm_qb(
                    pr, rall_store[pr], 2 * qb, qb
                )

        wo_sb = res.tile([P, NFT, D], BF16)
        yT3 = yT.rearrange("(o p) t -> p o t", p=P)

        def emit_outproj_group(ot, tb):
            ts = slice(tb * TB, (tb + 1) * TB)
            py = mmp.tile([P, TB], F32, tag="acc", name="py")
            for f in range(NFT):
                nc.tensor.matmul(
                    py[:], wo_sb[:, f, ot * P:(ot + 1) * P],
                    a_sb[:, f, ts],
                    start=(f == 0), stop=(f == NFT - 1),
                )
            yt = yst.tile([P, TB], BF16, tag="yt")
            nc.vector.tensor_copy(yt[:], py[:])
            nc.sync.dma_start(yT3[:, ot, ts], yt[:])

        def attention(pr, sched, qb_prologue=None):
            """sched: [(due_step, fn)] sorted by due-step; per kv-step all
            due units run (~1/step keeps a short PE burst between the AV pair
            and the next score pair so the score weight loads hide).  Scores
            run one step ahead of AV across qb boundaries."""
            last = pr == NFT - 1
            if not last:
                sums_p = sums.tile([8, TB], F32, tag="sums_p", name="sums_p")
                sums_ps[pr] = sums_p
            steps = [(qb, kv) for qb in range(NTB) for kv in range(nkv_of(qb))]
            kvstep = 0
            avs = {}
            s3s = {}
            pending = []  # scores run 2 kv-steps ahead of AV (exp+mask slack)

            def qb_end(qb):
                qs = slice(qb * TB, (qb + 1) * TB)
                av = avs.pop(qb)
                for hh in (0, 1):
                    srow = sums.tile([P, TB], F32, tag="srow", name="srow")
                    nc.vector.tensor_copy(
                        srow[DEPTH:DEPTH + 1, :], av[hh][DEPTH:DEPTH + 1, :]
                    )
                    if last:
                        nc.sync.dma_start(
                            s3s[qb][hh:hh + 1, :], srow[DEPTH:DEPTH + 1, :]
                        )
                    else:
                        nc.sync.dma_start(
                            sums_p[2 * qb + hh:2 * qb + hh + 1, :],
                            srow[DEPTH:DEPTH + 1, :],
                        )
                    nc.vector.tensor_copy(
                        a_sb[64 * hh:64 * hh + 64, pr, qs], av[hh][0:DEPTH, :]
                    )
                if last:
                    s3 = s3s.pop(qb)

                    rall3 = [None]

                    def norm3_part(part, qb=qb, s3=s3, rall3=rall3):
                        if part == 0:
                            rall3[0] = sums.tile([2, TB], F32R, tag="r3", name="rall3")
                        with nc.allow_low_precision(
                            reason="f32r holds fp32 bits; PE does the rounding"
                        ):
                            cs = slice(part * P, (part + 1) * P)
                            nc.vector.reciprocal(rall3[0][:, cs], s3[:, cs])
                        if part == TB // P - 1:
                            emit_norm_qb(pr, rall3[0], 0, qb)

                    parts = [lambda part=part: norm3_part(part)
                             for part in range(TB // P)]
                    if qb == NTB - 1 and deferred:
                        # final drain: interleave the DVE reciprocal chain
                        # with queued output-projection groups so the PE
                        # keeps streaming while the denominators resolve
                        head = []
                        for p in parts:
                            head.append(p)
                            if deferred:
                                head.append(deferred.pop(0))
                        deferred[:0] = head
                    else:
                        deferred.extend(parts)
                    for ot in range(D // P):
                        deferred.append(
                            lambda ot=ot, tb=qb: emit_outproj_group(ot, tb)
                        )

            def av_step(pqb, pkv, ppt, poff):
                for hh in (0, 1):
                    nc.tensor.matmul(
                        avs[pqb][hh][0:DEPTH + 1, poff:TB],
                        v_sb[:, pkv, 2 * pr + hh, :],
                        ppt[:, hh * TB + poff:(hh + 1) * TB],
                        start=(pkv == 0), stop=(pkv == nkv_of(pqb) - 1),
                    )

            for qb, kv in steps:
                if kv == 0:
                    if qb_prologue is not None:
                        qb_prologue(qb)
                    if last:
                        s3s[qb] = sums.tile([2, TB], F32, tag="s3", name="s3")
                # causal diagonal tiles: tokens below the kv tile can't
                # attend, so the score/exp/AV column window shrinks to
                # [i*P, TB) and only the 128-wide triangle block is masked
                di = kv - 4 * qb if (variant == "causal" and kv >= 4 * qb) else -1
                off = di * P if di > 0 else 0
                qs = slice(qb * TB + off, (qb + 1) * TB)
                sp = pssp.tile([P, 2 * TB], F32, tag="sp")
                for hh in (0, 1):
                    hs = slice(64 * hh, 64 * hh + 64)
                    nc.tensor.matmul(
                        sp[:, hh * TB + off:(hh + 1) * TB],
                        kt_sb[hs, pr, kv * P:(kv + 1) * P],
                        qt_sb[hs, pr, qs],
                        start=True, stop=True,
                    )
                pt = pp.tile([P, 2 * TB], BF16, tag="pt")
                if off:
                    for hh in (0, 1):
                        nc.scalar.activation(
                            pt[:, hh * TB + off:(hh + 1) * TB],
                            sp[:, hh * TB + off:(hh + 1) * TB],
                            mybir.ActivationFunctionType.Exp,
                            scale=float(SCALE),
                        )
                else:
                    nc.scalar.activation(
                        pt[:], sp[:], mybir.ActivationFunctionType.Exp,
                        scale=float(SCALE),
                    )
                # mask applied post-exp as a 0/1 multiply on the idle GpSimd
                # engine (SBUF-only), keeping the DVE off the exp->AV chain
                if di >= 0:
                    ms = slice(di * P, (di + 1) * P)
                    for hh in (0, 1):
                        nc.gpsimd.tensor_tensor(
                            pt[:, hh * TB + di * P:hh * TB + (di + 1) * P],
                            pt[:, hh * TB + di * P:hh * TB + (di + 1) * P],
                            mb_sb[:, di, ms], mybir.AluOpType.mult,
                        )
                elif variant == "general":
                    mg = mgp.tile([P, TB], BF16, tag="mg")
                    nc.sync.dma_start(mg[:], mb[:, kv, qb, :])
                    for hh in (0, 1):
                        nc.gpsimd.tensor_tensor(
                            pt[:, hh * TB:(hh + 1) * TB],
                            pt[:, hh * TB:(hh + 1) * TB],
                            mg[:], mybir.AluOpType.mult,
                        )
                pending.append((qb, kv, pt, off))
                if len(pending) > 2:
                    pqb, pkv, ppt, poff = pending.pop(0)
                    if pkv == 0:
                        avs[pqb] = [
                            psav.tile([P, TB], F32, tag=f"av{h}", name=f"av{h}")
                            for h in (0, 1)
                        ]
                    av_step(pqb, pkv, ppt, poff)
                    if pkv == nkv_of(pqb) - 1:
                        qb_end(pqb)
                kvstep += 1
                ran = False
                while sched and sched[0][0] <= kvstep:
                    sched.pop(0)[1]()
                    ran = True
                if deferred and not ran:
                    deferred.pop(0)()
                    if deferred and len(deferred) > 8:
                        deferred.pop(0)()
                elif deferred and len(deferred) > 24:
                    deferred.pop(0)()
            # drain the pipeline
            for pqb, pkv, ppt, poff in pending:
                if pkv == 0:
                    avs[pqb] = [
                        psav.tile([P, TB], F32, tag=f"av{h}", name=f"av{h}")
                        for h in (0, 1)
                    ]
                av_step(pqb, pkv, ppt, poff)
                if pkv == nkv_of(pqb) - 1:
                    qb_end(pqb)
            for _, u in sched:
                u()

        # ---- schedule ----------------------------------------------------
        deferred = []
        # Startup: xT arrives as 8 per-k-tile chunks round-robined over the
        # sync/scalar/vector DMA rings (k ascending); ft0's weights lead the
        # gpsimd ring so the first matmul can fire as soon as chunk 0 lands.
        fetch_w(0, ring=nc.gpsimd)
        # xt k-chunks across the three rings; per-ring transfers run
        # sequentially (~143 GB/s each), so chunk k's arrival time is its
        # queue position -- k0 is split in half across two rings so the
        # first projection matmuls can fire ~2us earlier
        nc.sync.dma_start(xt_sb[:, 0, 0:N // 2], xT3[:, 0, 0:N // 2])
        nc.scalar.dma_start(xt_sb[:, 0, N // 2:N], xT3[:, 0, N // 2:N])
        nc.sync.dma_start(xt_sb[:, 1, :], xT3[:, 1, :])
        nc.scalar.dma_start(xt_sb[:, 2, :], xT3[:, 2, :])
        nc.gpsimd.dma_start(xt_sb[:, 4, :], xT3[:, 4, :])
        nc.sync.dma_start(xt_sb[:, 3, :], xT3[:, 3, :])
        nc.scalar.dma_start(xt_sb[:, 5, :], xT3[:, 5, :])
        nc.gpsimd.dma_start(xt_sb[:, 7, :], xT3[:, 7, :])
        nc.sync.dma_start(xt_sb[:, 6, :], xT3[:, 6, :])
        nc.gpsimd.dma_start(wv_sb[:], wvT[:])
        if variant == "causal":
            nc.gpsimd.dma_start(mb_sb[:], mb[:])
        fetch_w(1, ring=nc.gpsimd)
        if not zb:
            nc.scalar.dma_start(bq_sb[:], bq2[:])
            nc.scalar.dma_start(bk_sb[:], bk2[:])
            nc.scalar.dma_start(ones_sb[:], ones_d[:])
            nc.scalar.dma_start(bv_sb[:], bv1[:])
        nc.scalar.dma_start(sel_sb[:], sel_d[:].bitcast(F32R))

        # ft0's eight Q/K projection groups run k-interleaved across all 8
        # PSUM banks so the PE tracks the xT chunk arrivals instead of
        # stalling on the full tensor.
        st_groups = [(tb, w) for w in ("q", "k") for tb in range(NTB)]
        st_acc = [mmp.tile([P, TB], F32, tag="acc", name="pqk0") for _ in range(2)]
        st_sp = [pssp.tile([P, 2 * TB], F32, tag="sp", name="sp0") for _ in range(2)]
        st_av = [psav.tile([P, TB], F32, tag=f"av{h}", name=f"av{h}0") for h in (0, 1)]
        st_slots = [
            st_acc[0][:, :], st_acc[1][:, :],
            st_sp[0][:, 0:TB], st_sp[0][:, TB:2 * TB],
            st_sp[1][:, 0:TB], st_sp[1][:, TB:2 * TB],
            st_av[0][:, :], st_av[1][:, :],
        ]
        korder = (0, 4, 1, 2, 3, 7, 5, 6)  # xt chunk DMA arrival order
        for ki, k in enumerate(korder):
            for (tb, which), ps in zip(st_groups, st_slots):
                w_sb = wq_sbs[0] if which == "q" else wk_sbs[0]
                nc.tensor.matmul(
                    ps, w_sb[:, k, :], xt_sb[:, k, tb * TB:(tb + 1) * TB],
                    start=(ki == 0), stop=(ki == KT - 1),
                )
        # drain in (tb, q/k) order so the first attention steps unblock first
        st_by_key = dict(zip(st_groups, st_slots))
        for tb in range(NTB):
            for which in ("q", "k"):
                qk_drain(0, tb, which, st_by_key[(tb, which)])

        def v_prologue(qb):
            # qb0's V tiles inline; later qbs' V groups ride the filler sched
            if variant == "causal":
                tts = range(4) if qb == 0 else ()
            else:
                tts = range(NTT) if qb == 0 else ()
            for tt in tts:
                emit_v_group(tt)

        ns = sum(nkv_of(qb) for qb in range(NTB))
        s0 = []
        if variant == "causal":
            for qb in (1, 2, 3):
                vu = []
                for tt in range(4 * qb, 4 * qb + 4):
                    vu += v_units(tt)
                # due before qb's steps begin (qb's AV starts one step in)
                d1 = sum(nkv_of(q) for q in range(qb))
                s0 += due_list(vu, d1 - 10 if qb > 1 else 0, d1)
            s0 += due_list(proj_units(1), 24, ns)
        else:
            s0 += due_list(proj_units(1), 4, ns)
        s0.sort(key=lambda t: t[0])
        attention(0, s0, qb_prologue=v_prologue)
        fetch_w(2)
        p2 = proj_units(2)
        s1 = due_list(p2, 1, ns) + due_list(list(norm_filler(0)), 4, 12)
        s1.sort(key=lambda t: t[0])
        attention(1, s1)
        fetch_w(3)
        p3 = proj_units(3)
        s2 = due_list(p3, 1, ns) + due_list(list(norm_filler(1)), 4, 12)
        s2.sort(key=lambda t: t[0])
        attention(2, s2)
        nc.sync.dma_start(wo_sb[:], woT[:])

        attention(3, due_list(list(norm_filler(2)), 4, 12))
        while deferred:
            deferred.pop(0)()

    _spill_excess_waits(nc)
    return nc


# ---------------------------------------------------------------------------
# Host side
# ---------------------------------------------------------------------------
_cache: dict[tuple, bass.Bass] = {}


def _get_program(variant: str, zb: bool) -> bass.Bass:
    key = (variant, zb)
    if key not in _cache:
        _cache[key] = build_program(variant, zb)
    return _cache[key]


def _mask_variant(mask: np.ndarray) -> str:
    if mask.all():
        return "full"
    if np.array_equal(mask, np.tril(np.ones_like(mask))):
        return "causal"
    return "general"


def _make_in_maps(input, mask, Wq, bq, Wk, bk, Wv, bv, Wo, bo, variant):
    input = np.asarray(input, np.float32)
    mask = np.asarray(mask, bool)
    Wq, Wk, Wv, Wo = (np.asarray(w, np.float32) for w in (Wq, Wk, Wv, Wo))
    bq, bk, bv = (np.asarray(b, np.float32) for b in (bq, bk, bv))
    sel = np.kron(np.eye(8, dtype=np.float32), np.ones((1, DEPTH), np.float32))

    mb_arrs = {}
    if variant != "full":
        # 0/1 multiplicative mask on P = exp(S^T) (applied post-exp)
        maskT01 = mask.T.astype(np.float32)
        if variant == "causal":
            # the diag-tile pattern only depends on kv-tile offset within the
            # 512-block, so 4 patterns cover all q blocks
            mb = np.empty((P, 4, TB), _BF16)
            for i in range(4):
                mb[:, i, :] = maskT01[i * P:(i + 1) * P, 0:TB]
        else:
            mb = (
                maskT01.reshape(NKV, P, NTB, TB)
                .transpose(1, 0, 2, 3)
                .astype(_BF16)
            )
        mb_arrs["mb"] = np.ascontiguousarray(mb)

    in_maps = []
    for c in range(NCORES):
        b, half = c // 2, c % 2
        fs = FH * half
        def tile_kp(wt):
            # [D, F] -> [P, KT, F] with row 128k+p -> [p, k]
            return wt.reshape(KT, P, -1).transpose(1, 0, 2)

        def tile_ft(wt):
            # [D, FH] -> [NFT, P, KT, P]: per f-tile, [p, k, f]
            return np.stack(
                [tile_kp(wt[:, ft * P:(ft + 1) * P]) for ft in range(NFT)]
            )

        m = {
            "xT": np.ascontiguousarray(tile_kp(input[b].T.astype(_BF16))),
            "wqT": np.ascontiguousarray(tile_ft(Wq[fs:fs + FH, :].T.astype(_BF16))),
            "wkT": np.ascontiguousarray(tile_ft(Wk[fs:fs + FH, :].T.astype(_BF16))),
            "wvT": np.ascontiguousarray(tile_kp(Wv[fs:fs + FH, :].T.astype(_BF16))),
            "woT": np.ascontiguousarray(
                Wo[:, fs:fs + FH].T.astype(_BF16).reshape(NFT, P, D).transpose(1, 0, 2)
            ),
            "bq2": np.ascontiguousarray(bq[fs:fs + FH].reshape(NFT, P).T),
            "bk2": np.ascontiguousarray(bk[fs:fs + FH].reshape(NFT, P).T),
            "bv1": np.ascontiguousarray(bv[fs:fs + FH].reshape(1, FH).astype(_BF16)),
            "ones": np.ones((P, P), _BF16),
            "sel": sel,
        }
        m.update(mb_arrs)
        in_maps.append(m)
    return in_maps


def _run(inputs: dict, trace: bool = False, tmpdir=None):
    from concourse.bass_utils import run_bass_kernel_spmd

    variant = _mask_variant(np.asarray(inputs["mask"], bool))
    zb = all(
        not np.any(np.asarray(inputs[k], np.float32))
        for k in ("bq", "bk", "bv")
    )
    nc = _get_program(variant, zb)
    in_maps = _make_in_maps(
        inputs["input"], inputs["mask"],
        inputs["Wq"], inputs["bq"], inputs["Wk"], inputs["bk"],
        inputs["Wv"], inputs["bv"], inputs["Wo"], inputs["bo"],
        variant,
    )
    res = run_bass_kernel_spmd(
        nc, in_maps, list(range(NCORES)), trace=trace, tmpdir=tmpdir
    )
    bo = np.asarray(inputs["bo"], np.float32)
    out = np.empty((B, N, D), np.float32)
    for b in range(B):
        yT = (
            res.results[2 * b]["yT"].astype(np.float32)
            + res.results[2 * b + 1]["yT"].astype(np.float32)
        )
        out[b] = yT.T + bo
    return out, res


def kernel(**inputs) -> np.ndarray:
    out, _ = _run(inputs, trace=False)
    return out



# revision 33
# speedup vs baseline: 1.0367x; 1.0161x over previous
"""Multi-head attention (B=4, N=2048, d_model=1024, 16 heads) on 8 trn2 cores.

Sharding: data-parallel over batch (4) x Megatron tensor-parallel over heads
(2-way column-split Wq/Wk/Wv, row-split Wo).  Core c handles batch c//2 and
heads [8*(c%2), 8*(c%2)+8).  Each core emits a partial Y^T [1024, 2048]; the
host sums core pairs, transposes, and adds the output bias.  No on-device
collectives (a 2-rank 8MB AllReduce costs more than the whole compute).

On-device pipeline per core (bulk matmuls in bf16, fp32 PSUM accumulate;
the softmax-denominator path stays float32r):
  Q^T,K^T [512,2048] and V [2048, 8x(64+1)] projections (V gets a ones column
  so the attention-weight row sums fall out of the AV matmul), then per head
  pair: S^T = K_h @ Q_h^T (K=64 contraction, two heads packed concurrently in
  the PE array via base partitions 0/64), exp on the scalar engine with the
  1/sqrt(64) scale folded in, 0/1 mask multiply post-exp on GpSimd (causal
  variant touches diagonal tiles only and skips upper-triangle tiles), AV
  matmul (lhsT = V_aug) giving A^T plus the softmax denominators, and a
  selector-matmul broadcast of the batched reciprocals to normalize.
  Y^T = WoT^T @ A^T at the end.

Scheduling: everything is software-pipelined by emission order (engines run
in-order): scores run one kv-step ahead of AV; the next pair's Q/K projection
groups, the previous pair's normalization, and (during the last pair) the
finished t-blocks of the output projection are injected into the attention
kv-loop as paced PE filler so the tensor engine never idles long enough for
the HAM clock gate to re-throttle.
"""

import sys

for _p in ("/opt/trn_rl_repo",):
    if _p not in sys.path:
        sys.path.insert(0, _p)

from contextlib import ExitStack

import ml_dtypes
import numpy as np

import concourse.bass as bass
import concourse.mybir as mybir
import concourse.tile as tile_mod
from concourse.vector_clock import ScopedClock

# ---------------------------------------------------------------------------
# Workaround: this walrus build rejects >1 sync wait on a Drain (CTRL_NO)
# instruction ("Too many sync wait commands").  Tile's end-of-context drain
# carries one wait per live processor, so redistribute the extras onto
# individual EventSemaphore wait instructions.
# ---------------------------------------------------------------------------


def _patched_drain_and_barrier(self, tick_clock, wait_clock):
    nc = self.nc
    drain_inst = nc.sync.drain()
    wait_clock.add_sem_waits(
        drain_inst.ins, ScopedClock({None: tick_clock.global_clock})
    )
    si = drain_inst.ins.sync_info
    waits = list(si.on_wait) if si is not None else []
    if len(waits) > 1:
        assert self.sems is not None
        num2handle = {h.num: h for h in self.sems.allocated().values()}
        drain_inst.ins.sync_info = mybir.SyncInfo(
            on_wait=[waits[0]], on_update=list(si.on_update)
        )
        for w in waits[1:]:
            h = num2handle.get(w.id)
            assert h is not None, f"no sem handle for {w.ant_name} (id {w.id})"
            assert w.wait_mode.startswith("sem-ge"), w.wait_mode
            nc.sync.wait_ge(h, w.wait_value)

    nc.all_engine_barrier()
    assert self.sems is not None
    popped = nc._tile_sem_poison_stack.pop()
    assert popped is self._sem_poison
    nc.clear_and_free_semaphores(list(self.sems.allocated().values()))
    nc.all_engine_barrier()


tile_mod.TileContext._drain_and_barrier = _patched_drain_and_barrier


def _spill_excess_waits(nc: bass.Bass) -> None:
    """This walrus build accepts at most 1 sync wait per instruction (2 for
    EventSemaphore).  Move excess waits onto EventSemaphore instructions
    inserted just before the over-subscribed instruction on the same engine."""
    n_new = 0
    for f in nc.m.functions:
        for blk in f.blocks:
            il = blk.instructions
            out = []
            changed = False
            for inst in il:
                si = inst.sync_info
                waits = list(si.on_wait) if si is not None else []
                cap = 2 if isinstance(inst, mybir.InstEventSemaphore) else 1
                if len(waits) > cap:
                    changed = True
                    extra, keep = waits[:-cap], waits[-cap:]
                    inst.sync_info = mybir.SyncInfo(
                        on_wait=keep, on_update=list(si.on_update)
                    )
                    for j in range(0, len(extra), 2):
                        n_new += 1
                        out.append(
                            mybir.InstEventSemaphore(
                                name=f"{inst.name}-xw{j}",
                                ins=[],
                                outs=[],
                                engine=inst.engine,
                                sync_info=mybir.SyncInfo(
                                    on_wait=extra[j:j + 2], on_update=[]
                                ),
                            )
                        )
                out.append(inst)
            if changed:
                il[:] = out

# ---------------------------------------------------------------------------
# Problem shapes (hardcoded per the task contract).
# ---------------------------------------------------------------------------
B, N, D = 4, 2048, 1024
NHEAD, DEPTH = 16, 64
NCORES = 8
FH = 512          # features per core (8 heads x 64)
HPC = 8           # heads per core
P = 128           # SBUF partitions
TB = 512          # token block (matmul moving free dim)
NTB = N // TB     # 4 token blocks
KT = D // P       # 8 contraction tiles for the projections
NFT = FH // P     # 4 feature tiles (= head pairs)
NTT = N // P      # 16 token tiles
NKV = N // P      # 16 kv tiles
SCALE = 1.0 / np.sqrt(DEPTH)
F32, F32R, BF16 = mybir.dt.float32, mybir.dt.float32r, mybir.dt.bfloat16

_BF16 = ml_dtypes.bfloat16


def build_program(variant: str, zb: bool = False) -> bass.Bass:
    """variant: 'causal' (tril mask), 'full' (all-true mask), 'general'.
    zb: all-zero q/k/v biases (skip bias loads + adds on device)."""
    assert variant in ("causal", "full", "general")
    nc = bass.Bass()

    # pre-tiled on the host: partition-major layouts for fast (contiguous
    # per-partition) DMA
    xT = nc.declare_dram_parameter("xT", [P, KT, N], BF16, isOutput=False)
    wqT = nc.declare_dram_parameter("wqT", [NFT, P, KT, P], BF16, isOutput=False)
    wkT = nc.declare_dram_parameter("wkT", [NFT, P, KT, P], BF16, isOutput=False)
    wvT = nc.declare_dram_parameter("wvT", [P, KT, FH], BF16, isOutput=False)
    woT = nc.declare_dram_parameter("woT", [P, NFT, D], BF16, isOutput=False)
    bq2 = nc.declare_dram_parameter("bq2", [P, NFT], F32, isOutput=False)
    bk2 = nc.declare_dram_parameter("bk2", [P, NFT], F32, isOutput=False)
    bv1 = nc.declare_dram_parameter("bv1", [1, FH], BF16, isOutput=False)
    ones_d = nc.declare_dram_parameter("ones", [P, P], BF16, isOutput=False)
    # block-diagonal selector for broadcasting 1/sums rows (f32r path)
    sel_d = nc.declare_dram_parameter("sel", [8, 8 * DEPTH], BF16, isOutput=False)
    if variant == "causal":
        # the 4 distinct diagonal-tile 0/1 patterns of the causal mask
        mb = nc.declare_dram_parameter("mb", [P, 4, TB], BF16, isOutput=False)
    elif variant == "general":
        mb = nc.declare_dram_parameter("mb", [P, NKV, NTB, TB], BF16, isOutput=False)
    # partials leave the device in bf16; the host sums the TP pair in f32
    yT = nc.declare_dram_parameter("yT", [D, N], BF16, isOutput=True)

    def nkv_of(qb):
        return 4 * (qb + 1) if variant == "causal" else NKV

    with tile_mod.TileContext(nc) as tc, ExitStack() as ctx:
        res = ctx.enter_context(tc.tile_pool(name="res", bufs=1))
        wp = ctx.enter_context(tc.tile_pool(name="w", bufs=2))
        pp = ctx.enter_context(tc.tile_pool(name="ppair", bufs=6))
        sums = ctx.enter_context(tc.tile_pool(name="sums", bufs=3))
        yst = ctx.enter_context(tc.tile_pool(name="yst", bufs=4))
        # PSUM: shared accumulator tag (2 banks) + score pair tiles (4) +
        # the two AV accumulators (2) = 8 banks exactly.
        mmp = ctx.enter_context(tc.tile_pool(name="mmp", bufs=2, space="PSUM"))
        pssp = ctx.enter_context(tc.tile_pool(name="pssp", bufs=2, space="PSUM"))
        psav = ctx.enter_context(tc.tile_pool(name="psav", bufs=1, space="PSUM"))
        if variant == "general":
            mgp = ctx.enter_context(tc.tile_pool(name="mg", bufs=4))

        if not zb:
            ones_sb = res.tile([P, P], BF16)
            bq_sb = res.tile([P, NFT], F32)
            bk_sb = res.tile([P, NFT], F32)
            bv_sb = res.tile([1, FH], BF16)
        sel_sb = res.tile([8, 8 * DEPTH], BF16)
        if variant == "causal":
            mb_sb = res.tile([P, 4, TB], BF16)

        qt_sb = res.tile([P, NFT, N], BF16)   # Q^T  [feat, tok]
        kt_sb = res.tile([P, NFT, N], BF16)   # K^T  [feat, tok]
        v_sb = res.tile([P, NTT, HPC, DEPTH + 1], BF16)  # V + ones col
        nc.gpsimd.memset(v_sb[:, :, :, DEPTH], 1.0)
        a_sb = res.tile([P, NFT, N], BF16)    # A^T (attention output)

        xt_sb = res.tile([P, KT, N], BF16)
        xT3 = xT
        wv_sb = res.tile([P, KT, FH], BF16)

        wq_sbs, wk_sbs = {}, {}

        def fetch_w(ft, ring=None):
            ring = ring or nc.sync
            wq_sbs[ft] = wp.tile([P, KT, P], BF16, tag="wq", name="wq_sb")
            wk_sbs[ft] = wp.tile([P, KT, P], BF16, tag="wk", name="wk_sb")
            ring.dma_start(wq_sbs[ft][:], wqT[ft])
            ring.dma_start(wk_sbs[ft][:], wkT[ft])

        def qk_drain(ft, tb, which, ps):
            ts = slice(tb * TB, (tb + 1) * TB)
            dst = qt_sb if which == "q" else kt_sb
            if zb:
                nc.vector.tensor_copy(dst[:, ft, ts], ps)
            else:
                bias = bq_sb if which == "q" else bk_sb
                nc.vector.tensor_tensor(
                    dst[:, ft, ts], ps,
                    bias[:, ft, None].to_broadcast((P, TB)),
                    mybir.AluOpType.add,
                )

        def emit_qk_group(ft, tb, which):
            ts = slice(tb * TB, (tb + 1) * TB)
            w_sb = wq_sbs[ft] if which == "q" else wk_sbs[ft]
            ps = mmp.tile([P, TB], F32, tag="acc", name="pqk")
            for k in range(KT):
                nc.tensor.matmul(
                    ps[:], w_sb[:, k, :], xt_sb[:, k, ts],
                    start=(k == 0), stop=(k == KT - 1),
                )
            qk_drain(ft, tb, which, ps[:])

        def qk_units(ft, tb, which):
            """One Q/K projection group split into 4 two-matmul filler units
            (shared PSUM accumulator; the last unit drains to SBUF)."""
            st = {}

            def u(j, ft=ft, tb=tb, which=which):
                if j == 0:
                    st["ps"] = mmp.tile([P, TB], F32, tag="acc", name="pqk")
                ps = st["ps"]
                w_sb = wq_sbs[ft] if which == "q" else wk_sbs[ft]
                ts = slice(tb * TB, (tb + 1) * TB)
                for k in (2 * j, 2 * j + 1):
                    nc.tensor.matmul(
                        ps[:], w_sb[:, k, :], xt_sb[:, k, ts],
                        start=(k == 0), stop=(k == KT - 1),
                    )
                if j == 3:
                    qk_drain(ft, tb, which, ps[:])

            return [lambda j=j: u(j) for j in range(4)]

        def proj_units(ft):
            out = []
            for tb in range(NTB):
                for which in ("q", "k"):
                    out += qk_units(ft, tb, which)
            return out

        def v_units(tt):
            """One V projection group split into 4 two-matmul units."""
            st = {}

            def u(j, tt=tt):
                if j == 0:
                    st["pv"] = mmp.tile([P, TB], F32, tag="acc", name="pv")
                pv = st["pv"]
                for k in (2 * j, 2 * j + 1):
                    nc.tensor.matmul(
                        pv[:], xt_sb[:, k, tt * P:(tt + 1) * P], wv_sb[:, k, :],
                        start=(k == 0), stop=(zb and k == KT - 1),
                    )
                if j == 3:
                    if not zb:
                        nc.tensor.matmul(
                            pv[:], ones_sb[0:1, 0:P], bv_sb[0:1, :],
                            start=False, stop=True,
                        )
                    nc.vector.tensor_copy(
                        v_sb[:, tt, :, 0:DEPTH],
                        pv[:].rearrange("p (h d) -> p h d", h=HPC),
                    )

            return [lambda j=j: u(j) for j in range(4)]

        def due_list(units, d0, d1):
            """Spread units evenly over due-steps [d0, d1)."""
            m = max(1, len(units))
            return [
                (d0 + (i * max(1, d1 - d0)) // m, u)
                for i, u in enumerate(units)
            ]

        def emit_v_group(tt):
            pv = mmp.tile([P, TB], F32, tag="acc", name="pv")
            for k in range(KT):
                nc.tensor.matmul(
                    pv[:], xt_sb[:, k, tt * P:(tt + 1) * P], wv_sb[:, k, :],
                    start=(k == 0), stop=(zb and k == KT - 1),
                )
            if not zb:
                nc.tensor.matmul(
                    pv[:], ones_sb[0:1, 0:P], bv_sb[0:1, :],
                    start=False, stop=True,
                )
            nc.vector.tensor_copy(
                v_sb[:, tt, :, 0:DEPTH],
                pv[:].rearrange("p (h d) -> p h d", h=HPC),
            )

        def emit_norm_qb(pr, rall, row0, qb, bc_ap=None):
            """bc2 = broadcast of 1/sums rows (2qb+hh) over the head depth;
            one in-place multiply normalizes both heads of the pair."""
            qs = slice(qb * TB, (qb + 1) * TB)
            nrows = rall.shape[0]
            if bc_ap is None:
                bc_ap = mmp.tile([P, TB], F32, tag="acc", name="bc")[:]
            nc.tensor.matmul(
                bc_ap,
                sel_sb[0:nrows, row0 * DEPTH:(row0 + 2) * DEPTH],
                rall[:],
                start=True, stop=True,
            )
            nc.vector.tensor_tensor(
                a_sb[:, pr, qs], a_sb[:, pr, qs], bc_ap,
                mybir.AluOpType.mult,
            )

        sums_ps = {}
        rall_store = {}

        def norm_filler(pr):
            def recip(pr=pr, part=0):
                if part == 0:
                    rall_store[pr] = sums.tile([8, TB], BF16, tag="rall", name="rall")
                with nc.allow_low_precision(
                    reason="f32r holds fp32 bits; rounding happens in the PE"
                ):
                    cs = slice(part * P, (part + 1) * P)
                    nc.vector.reciprocal(
                        rall_store[pr][:, cs], sums_ps[pr][:, cs]
                    )

            for part in range(TB // P):
                yield lambda pr=pr, part=part: recip(pr, part)
            for qb in range(NTB):
                yield lambda pr=pr, qb=qb: emit_norm_qb(
                    pr, rall_store[pr], 2 * qb, qb
                )

        wo_sb = res.tile([P, NFT, D], BF16)
        yT3 = yT.rearrange("(o p) t -> p o t", p=P)

        def emit_outproj_group(ot, tb):
            ts = slice(tb * TB, (tb + 1) * TB)
            py = mmp.tile([P, TB], F32, tag="acc", name="py")
            for f in range(NFT):
                nc.tensor.matmul(
                    py[:], wo_sb[:, f, ot * P:(ot + 1) * P],
                    a_sb[:, f, ts],
                    start=(f == 0), stop=(f == NFT - 1),
                )
            yt = yst.tile([P, TB], BF16, tag="yt")
            nc.vector.tensor_copy(yt[:], py[:])
            nc.sync.dma_start(yT3[:, ot, ts], yt[:])

        def attention(pr, sched, qb_prologue=None):
            """sched: [(due_step, fn)] sorted by due-step; per kv-step all
            due units run (~1/step keeps a short PE burst between the AV pair
            and the next score pair so the score weight loads hide).  Scores
            run one step ahead of AV across qb boundaries."""
            last = pr == NFT - 1
            if not last:
                sums_p = sums.tile([8, TB], F32, tag="sums_p", name="sums_p")
                sums_ps[pr] = sums_p
            steps = [(qb, kv) for qb in range(NTB) for kv in range(nkv_of(qb))]
            kvstep = 0
            avs = {}
            s3s = {}
            pending = []  # scores run 2 kv-steps ahead of AV (exp+mask slack)

            def qb_end(qb):
                qs = slice(qb * TB, (qb + 1) * TB)
                av = avs.pop(qb)
                for hh in (0, 1):
                    srow = sums.tile([P, TB], F32, tag="srow", name="srow")
                    nc.vector.tensor_copy(
                        srow[DEPTH:DEPTH + 1, :], av[hh][DEPTH:DEPTH + 1, :]
                    )
                    if last:
                        nc.sync.dma_start(
                            s3s[qb][hh:hh + 1, :], srow[DEPTH:DEPTH + 1, :]
                        )
                    else:
                        nc.sync.dma_start(
                            sums_p[2 * qb + hh:2 * qb + hh + 1, :],
                            srow[DEPTH:DEPTH + 1, :],
                        )
                    nc.vector.tensor_copy(
                        a_sb[64 * hh:64 * hh + 64, pr, qs], av[hh][0:DEPTH, :]
                    )
                if last:
                    s3 = s3s.pop(qb)

                    rall3 = [None]

                    def norm3_part(part, qb=qb, s3=s3, rall3=rall3, bc_ap=None):
                        if part == 0:
                            rall3[0] = sums.tile([2, TB], BF16, tag="r3", name="rall3")
                        with nc.allow_low_precision(
                            reason="f32r holds fp32 bits; PE does the rounding"
                        ):
                            cs = slice(part * P, (part + 1) * P)
                            nc.vector.reciprocal(rall3[0][:, cs], s3[:, cs])
                        if part == TB // P - 1:
                            emit_norm_qb(pr, rall3[0], 0, qb, bc_ap=bc_ap)

                    if qb == NTB - 1:
                        # Final drain: open all 8 output-projection
                        # accumulators across every PSUM bank and run the
                        # f<3 contractions immediately, so the in-order PE
                        # queue streams while the qb3 denominators resolve
                        # on the DVE; then close each with the f=3 matmul.
                        qs3 = slice(qb * TB, (qb + 1) * TB)
                        fa = [mmp.tile([P, TB], F32, tag="acc", name="fpy")
                              for _ in range(2)]
                        fs = [pssp.tile([P, 2 * TB], F32, tag="sp", name="fsp")
                              for _ in range(2)]
                        fv0 = psav.tile([P, TB], F32, tag="av0", name="fav0")
                        slots = [
                            fa[0][:, :], fa[1][:, :],
                            fs[0][:, 0:TB], fs[0][:, TB:2 * TB],
                            fs[1][:, 0:TB], fs[1][:, TB:2 * TB],
                            fv0[:, :],
                        ]
                        for ot in range(D // P - 1):
                            for f in range(NFT - 1):
                                nc.tensor.matmul(
                                    slots[ot],
                                    wo_sb[:, f, ot * P:(ot + 1) * P],
                                    a_sb[:, f, qs3],
                                    start=(f == 0), stop=False,
                                )
                        # denominators resolve on the DVE while the PE
                        # streams the f<3 contractions above; the broadcast
                        # matmul gets the one PSUM bank the slots don't own
                        bc3 = psav.tile([P, TB], F32, tag="av1", name="bc3")
                        for part in range(TB // P):
                            norm3_part(part, bc_ap=bc3[:])
                        for ot in range(D // P - 1):
                            nc.tensor.matmul(
                                slots[ot],
                                wo_sb[:, NFT - 1, ot * P:(ot + 1) * P],
                                a_sb[:, NFT - 1, qs3],
                                start=False, stop=True,
                            )
                            yt = yst.tile([P, TB], BF16, tag="yt")
                            nc.vector.tensor_copy(yt[:], slots[ot])
                            nc.sync.dma_start(yT3[:, ot, qs3], yt[:])
                        ot7 = D // P - 1
                        p7 = psav.tile([P, TB], F32, tag="av1", name="py7")
                        for f in range(NFT):
                            nc.tensor.matmul(
                                p7[:], wo_sb[:, f, ot7 * P:(ot7 + 1) * P],
                                a_sb[:, f, qs3],
                                start=(f == 0), stop=(f == NFT - 1),
                            )
                        yt = yst.tile([P, TB], BF16, tag="yt")
                        nc.vector.tensor_copy(yt[:], p7[:])
                        nc.sync.dma_start(yT3[:, ot7, qs3], yt[:])
                    else:
                        for part in range(TB // P):
                            deferred.append(lambda part=part: norm3_part(part))
                        for ot in range(D // P):
                            deferred.append(
                                lambda ot=ot, tb=qb: emit_outproj_group(ot, tb)
                            )

            def av_step(pqb, pkv, ppt, poff):
                for hh in (0, 1):
                    nc.tensor.matmul(
                        avs[pqb][hh][0:DEPTH + 1, poff:TB],
                        v_sb[:, pkv, 2 * pr + hh, :],
                        ppt[:, hh * TB + poff:(hh + 1) * TB],
                        start=(pkv == 0), stop=(pkv == nkv_of(pqb) - 1),
                    )

            for qb, kv in steps:
                if kv == 0:
                    if qb_prologue is not None:
                        qb_prologue(qb)
                    if last:
                        s3s[qb] = sums.tile([2, TB], F32, tag="s3", name="s3")
                # causal diagonal tiles: tokens below the kv tile can't
                # attend, so the score/exp/AV column window shrinks to
                # [i*P, TB) and only the 128-wide triangle block is masked
                di = kv - 4 * qb if (variant == "causal" and kv >= 4 * qb) else -1
                off = di * P if di > 0 else 0
                qs = slice(qb * TB + off, (qb + 1) * TB)
                sp = pssp.tile([P, 2 * TB], F32, tag="sp")
                for hh in (0, 1):
                    hs = slice(64 * hh, 64 * hh + 64)
                    nc.tensor.matmul(
                        sp[:, hh * TB + off:(hh + 1) * TB],
                        kt_sb[hs, pr, kv * P:(kv + 1) * P],
                        qt_sb[hs, pr, qs],
                        start=True, stop=True,
                    )
                pt = pp.tile([P, 2 * TB], BF16, tag="pt")
                if off:
                    for hh in (0, 1):
                        nc.scalar.activation(
                            pt[:, hh * TB + off:(hh + 1) * TB],
                            sp[:, hh * TB + off:(hh + 1) * TB],
                            mybir.ActivationFunctionType.Exp,
                            scale=float(SCALE),
                        )
                else:
                    nc.scalar.activation(
                        pt[:], sp[:], mybir.ActivationFunctionType.Exp,
                        scale=float(SCALE),
                    )
                # mask applied post-exp as a 0/1 multiply on the idle GpSimd
                # engine (SBUF-only), keeping the DVE off the exp->AV chain
                if di >= 0:
                    ms = slice(di * P, (di + 1) * P)
                    for hh in (0, 1):
                        nc.gpsimd.tensor_tensor(
                            pt[:, hh * TB + di * P:hh * TB + (di + 1) * P],
                            pt[:, hh * TB + di * P:hh * TB + (di + 1) * P],
                            mb_sb[:, di, ms], mybir.AluOpType.mult,
                        )
                elif variant == "general":
                    mg = mgp.tile([P, TB], BF16, tag="mg")
                    nc.sync.dma_start(mg[:], mb[:, kv, qb, :])
                    for hh in (0, 1):
                        nc.gpsimd.tensor_tensor(
                            pt[:, hh * TB:(hh + 1) * TB],
                            pt[:, hh * TB:(hh + 1) * TB],
                            mg[:], mybir.AluOpType.mult,
                        )
                pending.append((qb, kv, pt, off))
                if len(pending) > 2:
                    pqb, pkv, ppt, poff = pending.pop(0)
                    if pkv == 0:
                        avs[pqb] = [
                            psav.tile([P, TB], F32, tag=f"av{h}", name=f"av{h}")
                            for h in (0, 1)
                        ]
                    av_step(pqb, pkv, ppt, poff)
                    if pkv == nkv_of(pqb) - 1:
                        qb_end(pqb)
                kvstep += 1
                ran = False
                while sched and sched[0][0] <= kvstep:
                    sched.pop(0)[1]()
                    ran = True
                if deferred and not ran:
                    deferred.pop(0)()
                    if deferred and len(deferred) > 8:
                        deferred.pop(0)()
                elif deferred and len(deferred) > 24:
                    deferred.pop(0)()
            # drain the pipeline
            for pqb, pkv, ppt, poff in pending:
                if pkv == 0:
                    avs[pqb] = [
                        psav.tile([P, TB], F32, tag=f"av{h}", name=f"av{h}")
                        for h in (0, 1)
                    ]
                av_step(pqb, pkv, ppt, poff)
                if pkv == nkv_of(pqb) - 1:
                    qb_end(pqb)
            for _, u in sched:
                u()

        # ---- schedule ----------------------------------------------------
        deferred = []
        # Startup: xT arrives as 8 per-k-tile chunks round-robined over the
        # sync/scalar/vector DMA rings (k ascending); ft0's weights lead the
        # gpsimd ring so the first matmul can fire as soon as chunk 0 lands.
        fetch_w(0, ring=nc.gpsimd)
        # xt k-chunks across the three rings; per-ring transfers run
        # sequentially (~143 GB/s each), so chunk k's arrival time is its
        # queue position -- k0 is split in half across two rings so the
        # first projection matmuls can fire ~2us earlier
        nc.sync.dma_start(xt_sb[:, 0, 0:N // 2], xT3[:, 0, 0:N // 2])
        nc.scalar.dma_start(xt_sb[:, 0, N // 2:N], xT3[:, 0, N // 2:N])
        nc.sync.dma_start(xt_sb[:, 1, :], xT3[:, 1, :])
        nc.scalar.dma_start(xt_sb[:, 2, :], xT3[:, 2, :])
        nc.gpsimd.dma_start(xt_sb[:, 4, :], xT3[:, 4, :])
        nc.sync.dma_start(xt_sb[:, 3, :], xT3[:, 3, :])
        nc.scalar.dma_start(xt_sb[:, 5, :], xT3[:, 5, :])
        nc.gpsimd.dma_start(xt_sb[:, 7, :], xT3[:, 7, :])
        nc.sync.dma_start(xt_sb[:, 6, :], xT3[:, 6, :])
        nc.gpsimd.dma_start(wv_sb[:], wvT[:])
        if variant == "causal":
            nc.gpsimd.dma_start(mb_sb[:], mb[:])
        fetch_w(1, ring=nc.gpsimd)
        if not zb:
            nc.scalar.dma_start(bq_sb[:], bq2[:])
            nc.scalar.dma_start(bk_sb[:], bk2[:])
            nc.scalar.dma_start(ones_sb[:], ones_d[:])
            nc.scalar.dma_start(bv_sb[:], bv1[:])
        nc.scalar.dma_start(sel_sb[:], sel_d[:])

        # ft0's eight Q/K projection groups run k-interleaved across all 8
        # PSUM banks so the PE tracks the xT chunk arrivals instead of
        # stalling on the full tensor.
        st_groups = [(tb, w) for w in ("q", "k") for tb in range(NTB)]
        st_acc = [mmp.tile([P, TB], F32, tag="acc", name="pqk0") for _ in range(2)]
        st_sp = [pssp.tile([P, 2 * TB], F32, tag="sp", name="sp0") for _ in range(2)]
        st_av = [psav.tile([P, TB], F32, tag=f"av{h}", name=f"av{h}0") for h in (0, 1)]
        st_slots = [
            st_acc[0][:, :], st_acc[1][:, :],
            st_sp[0][:, 0:TB], st_sp[0][:, TB:2 * TB],
            st_sp[1][:, 0:TB], st_sp[1][:, TB:2 * TB],
            st_av[0][:, :], st_av[1][:, :],
        ]
        korder = (0, 4, 1, 2, 3, 7, 5, 6)  # xt chunk DMA arrival order
        for ki, k in enumerate(korder):
            for (tb, which), ps in zip(st_groups, st_slots):
                w_sb = wq_sbs[0] if which == "q" else wk_sbs[0]
                nc.tensor.matmul(
                    ps, w_sb[:, k, :], xt_sb[:, k, tb * TB:(tb + 1) * TB],
                    start=(ki == 0), stop=(ki == KT - 1),
                )
        # drain in (tb, q/k) order so the first attention steps unblock first
        st_by_key = dict(zip(st_groups, st_slots))
        for tb in range(NTB):
            for which in ("q", "k"):
                qk_drain(0, tb, which, st_by_key[(tb, which)])

        def v_prologue(qb):
            # qb0's V tiles inline; later qbs' V groups ride the filler sched
            if variant == "causal":
                tts = range(4) if qb == 0 else ()
            else:
                tts = range(NTT) if qb == 0 else ()
            for tt in tts:
                emit_v_group(tt)

        ns = sum(nkv_of(qb) for qb in range(NTB))
        s0 = []
        if variant == "causal":
            for qb in (1, 2, 3):
                vu = []
                for tt in range(4 * qb, 4 * qb + 4):
                    vu += v_units(tt)
                # due before qb's steps begin (qb's AV starts one step in)
                d1 = sum(nkv_of(q) for q in range(qb))
                s0 += due_list(vu, d1 - 10 if qb > 1 else 0, d1)
            s0 += due_list(proj_units(1), 24, ns)
        else:
            s0 += due_list(proj_units(1), 4, ns)
        s0.sort(key=lambda t: t[0])
        attention(0, s0, qb_prologue=v_prologue)
        fetch_w(2)
        p2 = proj_units(2)
        s1 = due_list(p2, 1, ns) + due_list(list(norm_filler(0)), 4, 12)
        s1.sort(key=lambda t: t[0])
        attention(1, s1)
        fetch_w(3)
        p3 = proj_units(3)
        s2 = due_list(p3, 1, ns) + due_list(list(norm_filler(1)), 4, 12)
        s2.sort(key=lambda t: t[0])
        attention(2, s2)
        nc.sync.dma_start(wo_sb[:], woT[:])

        attention(3, due_list(list(norm_filler(2)), 4, 12))
        while deferred:
            deferred.pop(0)()

    _spill_excess_waits(nc)
    return nc


# ---------------------------------------------------------------------------
# Host side
# ---------------------------------------------------------------------------
_cache: dict[tuple, bass.Bass] = {}


def _get_program(variant: str, zb: bool) -> bass.Bass:
    key = (variant, zb)
    if key not in _cache:
        _cache[key] = build_program(variant, zb)
    return _cache[key]


def _mask_variant(mask: np.ndarray) -> str:
    if mask.all():
        return "full"
    if np.array_equal(mask, np.tril(np.ones_like(mask))):
        return "causal"
    return "general"


def _make_in_maps(input, mask, Wq, bq, Wk, bk, Wv, bv, Wo, bo, variant):
    input = np.asarray(input, np.float32)
    mask = np.asarray(mask, bool)
    Wq, Wk, Wv, Wo = (np.asarray(w, np.float32) for w in (Wq, Wk, Wv, Wo))
    bq, bk, bv = (np.asarray(b, np.float32) for b in (bq, bk, bv))
    sel = np.kron(np.eye(8, dtype=_BF16), np.ones((1, DEPTH), _BF16))

    mb_arrs = {}
    if variant != "full":
        # 0/1 multiplicative mask on P = exp(S^T) (applied post-exp)
        maskT01 = mask.T.astype(np.float32)
        if variant == "causal":
            # the diag-tile pattern only depends on kv-tile offset within the
            # 512-block, so 4 patterns cover all q blocks
            mb = np.empty((P, 4, TB), _BF16)
            for i in range(4):
                mb[:, i, :] = maskT01[i * P:(i + 1) * P, 0:TB]
        else:
            mb = (
                maskT01.reshape(NKV, P, NTB, TB)
                .transpose(1, 0, 2, 3)
                .astype(_BF16)
            )
        mb_arrs["mb"] = np.ascontiguousarray(mb)

    in_maps = []
    for c in range(NCORES):
        b, half = c // 2, c % 2
        fs = FH * half
        def tile_kp(wt):
            # [D, F] -> [P, KT, F] with row 128k+p -> [p, k]
            return wt.reshape(KT, P, -1).transpose(1, 0, 2)

        def tile_ft(wt):
            # [D, FH] -> [NFT, P, KT, P]: per f-tile, [p, k, f]
            return np.stack(
                [tile_kp(wt[:, ft * P:(ft + 1) * P]) for ft in range(NFT)]
            )

        m = {
            "xT": np.ascontiguousarray(tile_kp(input[b].T.astype(_BF16))),
            "wqT": np.ascontiguousarray(tile_ft(Wq[fs:fs + FH, :].T.astype(_BF16))),
            "wkT": np.ascontiguousarray(tile_ft(Wk[fs:fs + FH, :].T.astype(_BF16))),
            "wvT": np.ascontiguousarray(tile_kp(Wv[fs:fs + FH, :].T.astype(_BF16))),
            "woT": np.ascontiguousarray(
                Wo[:, fs:fs + FH].T.astype(_BF16).reshape(NFT, P, D).transpose(1, 0, 2)
            ),
            "bq2": np.ascontiguousarray(bq[fs:fs + FH].reshape(NFT, P).T),
            "bk2": np.ascontiguousarray(bk[fs:fs + FH].reshape(NFT, P).T),
            "bv1": np.ascontiguousarray(bv[fs:fs + FH].reshape(1, FH).astype(_BF16)),
            "ones": np.ones((P, P), _BF16),
            "sel": sel,
        }
        m.update(mb_arrs)
        in_maps.append(m)
    return in_maps


def _run(inputs: dict, trace: bool = False, tmpdir=None):
    from concourse.bass_utils import run_bass_kernel_spmd

    variant = _mask_variant(np.asarray(inputs["mask"], bool))
    zb = all(
        not np.any(np.asarray(inputs[k], np.float32))
        for k in ("bq", "bk", "bv")
    )
    nc = _get_program(variant, zb)
    in_maps = _make_in_maps(
        inputs["input"], inputs["mask"],
        inputs["Wq"], inputs["bq"], inputs["Wk"], inputs["bk"],
        inputs["Wv"], inputs["bv"], inputs["Wo"], inputs["bo"],
        variant,
    )
    res = run_bass_kernel_spmd(
        nc, in_maps, list(range(NCORES)), trace=trace, tmpdir=tmpdir
    )
    bo = np.asarray(inputs["bo"], np.float32)
    out = np.empty((B, N, D), np.float32)
    for b in range(B):
        yT = (
            res.results[2 * b]["yT"].astype(np.float32)
            + res.results[2 * b + 1]["yT"].astype(np.float32)
        )
        out[b] = yT.T + bo
    return out, res


def kernel(**inputs) -> np.ndarray:
    out, _ = _run(inputs, trace=False)
    return out

